# revision 13
# baseline (speedup 1.0000x reference)
"""Single-head attention kernel for Trainium2, SPMD across 8 NeuronCores.

Problem: x [4, 4096, 1024], Wq/Wk/Wv [128, 1024] ->
  q/k/v = x @ W.T ; scores = q k^T * 1024**-0.5 ; out = softmax(scores) @ v
Output: [4, 4096, 128] fp32.

Sharding: batch b = core//2, query half = core%2. Each core receives its
batch's x fully (transposed, with the sequence rolled so its own query
half occupies rows 0..2047 -- softmax over keys is permutation invariant,
so rolling keys/values together is harmless), computes K/V for all 4096
keys and attention outputs for its 2048 queries.

Kernel layout (per core):
  xT   [1024, 4096]  (d-major)     Wt   [1024, 128] per projection
  Q^T [h=128, n=2048] = sum_c Wt[c].T @ xT[c]        (c: 8 chunks of d)
  K^T [h=128, m=4096], V^T likewise; V^T is PE-transposed to V [m, h]
  S^T [m-tile=128, n 512-chunk] = (K^T m-slice).T @ Q^T n-slice
  P^T = exp(S^T / 32)  (scores are in [-1, 1]: max-subtraction skipped)
  O^T [h, n] += V[m-tile].T @ P^T ; rowsum[1, n] += ones.T @ P^T
  O = transpose(O^T) * (1 / transpose(rowsum))  -> dram
"""

import numpy as np
import ml_dtypes

import concourse.bass as bass
import concourse.mybir as mybir
import concourse.tile as tile
from concourse import bacc
from concourse.bass_utils import run_bass_kernel_spmd
from concourse.masks import make_identity

B, N, D, H = 4, 4096, 1024, 128
NCORES = 8
NQ = N // 2          # queries per core
DC = D // 128        # 8 contraction chunks
NCH = NQ // 512      # 4 query chunks of 512
MT = N // 128        # 32 key tiles
SCALE = float(D) ** -0.5

BF = mybir.dt.bfloat16
F32 = mybir.dt.float32
NPBF = ml_dtypes.bfloat16


def _build():
    nc = bacc.Bacc(None, target_bir_lowering=False, debug=True)

    xT = nc.declare_dram_parameter("xT", [D, N], BF, isOutput=False)
    wqT = nc.declare_dram_parameter("wqT", [D, H], BF, isOutput=False)
    wkT = nc.declare_dram_parameter("wkT", [D, H], BF, isOutput=False)
    wvT = nc.declare_dram_parameter("wvT", [D, H], BF, isOutput=False)
    out = nc.declare_dram_parameter("out", [NQ, H], F32, isOutput=True)

    xT_t = xT.rearrange("(c p) m -> c p m", p=128)
    w_ts = [w.rearrange("(c p) h -> c p h", p=128) for w in (wqT, wkT, wvT)]

    with tile.TileContext(nc) as tc:
        with (
            tc.tile_pool(name="const", bufs=1) as const,
            tc.tile_pool(name="sb", bufs=1) as sb,
            tc.tile_pool(name="vt_tmp_pool", bufs=2) as vt_pool,
            tc.tile_pool(name="p_pool", bufs=3) as p_pool,
            tc.tile_pool(name="epi", bufs=2) as epi,
            tc.tile_pool(name="outp", bufs=3) as outp,
            tc.tile_pool(name="ps", bufs=2, space="PSUM") as ps,
        ):
            # ---- constants / persistent SBUF ----
            xt = sb.tile([128, DC, N], BF)
            wq = sb.tile([128, DC, H], BF, name="wq")
            wk = sb.tile([128, DC, H], BF, name="wk")
            wv = sb.tile([128, DC, H], BF, name="wv")
            qT = sb.tile([128, NQ], BF)
            kT = sb.tile([128, N], BF)
            vv = sb.tile([128, MT, H], BF)

            ident_bf = const.tile([128, 128], BF)
            make_identity(nc, ident_bf)
            ident32 = const.tile([128, 128], F32)
            make_identity(nc, ident32)
            ones = const.tile([128, 1], BF)
            nc.vector.memset(ones[:], 1.0)
            rpad = const.tile([128, 512], F32)
            nc.vector.memset(rpad[:], 0.0)

            # ---- loads (interleaved so chunk c's weights+x arrive together) ----
            for c in range(DC):
                for wtile, wdram in zip((wq, wk, wv), w_ts):
                    nc.sync.dma_start(out=wtile[:, c, :], in_=wdram[c])
                nc.sync.dma_start(out=xt[:, c, :N // 2], in_=xT_t[c][:, :N // 2])
                nc.sync.dma_start(out=xt[:, c, N // 2:], in_=xT_t[c][:, N // 2:])

            # ---- projections: Q^T, K^T, and V (via V^T + PE transpose) ----
            # Two passes over the contraction (d chunks 0-3, then 4-7) with an
            # fp32 SBUF staging buffer between them, so matmuls for pass A can
            # run as soon as the first half of x arrives instead of every
            # projection waiting for the full 8 MB load.
            qs = sb.tile([128, NQ], F32, name="qs")
            ks = sb.tile([128, N], F32, name="ks")
            vs = sb.tile([128, N], F32, name="vs")
            # Sub-passes of 2 contraction chunks each: PSUM-accumulate the
            # pair, then fold into the fp32 staging buffer on the DVE.  Each
            # sub-pass only needs 2 x-chunks resident, so projection matmuls
            # track the x DMA stream instead of waiting for the whole load.
            proj_tiles = (
                [("q", wq, j) for j in range(NCH)]
                + [("k", wk, j) for j in range(N // 512)]
                + [("v", wv, j) for j in range(N // 512)]
            )
            stage_of = {"q": qs, "k": ks, "v": vs}
            for sp in range(DC // 2):
                c0, c1 = 2 * sp, 2 * sp + 1
                last = sp == DC // 2 - 1
                for which, wtile, j in proj_tiles:
                    sl = slice(j * 512, (j + 1) * 512)
                    psa = ps.tile([128, 512], F32, tag="t", name="psa")
                    nc.tensor.matmul(
                        psa[:], wtile[:, c0, :], xt[:, c0, sl],
                        start=True, stop=False,
                    )
                    nc.tensor.matmul(
                        psa[:], wtile[:, c1, :], xt[:, c1, sl],
                        start=False, stop=True,
                    )
                    stage = stage_of[which]
                    if sp == 0:
                        nc.vector.tensor_copy(stage[:, sl], psa[:])
                    elif not last:
                        nc.vector.tensor_add(stage[:, sl], psa[:], stage[:, sl])
                    elif which == "q":
                        nc.vector.tensor_add(qT[:, sl], psa[:], qs[:, sl])
                    elif which == "k":
                        nc.vector.tensor_add(kT[:, sl], psa[:], ks[:, sl])
                    else:
                        vt_tmp = vt_pool.tile([128, 512], BF)
                        nc.vector.tensor_add(vt_tmp[:], psa[:], vs[:, sl])
                        for t in range(4):
                            mt = j * 4 + t
                            psvt = ps.tile([128, 128], BF, tag="t", name="psvt")
                            nc.tensor.transpose(
                                psvt[:], vt_tmp[:, t * 128:(t + 1) * 128], ident_bf[:]
                            )
                            nc.vector.tensor_copy(vv[:, mt, :], psvt[:])

            # ---- attention (software-pipelined: S runs 2 pairs ahead) ----
            NP = MT // 2
            steps = [(j, p) for j in range(NCH) for p in range(NP)]
            pT_of = {}
            pso_of = {}
            psr_of = {}

            def emit_s(j, p):
                nsl = slice(j * 512, (j + 1) * 512)
                mt0, mt1 = 2 * p, 2 * p + 1
                pss = ps.tile([128, 1024], F32, tag="s", name="pss")
                nc.tensor.matmul(
                    pss[:, :512], kT[:, mt0 * 128:(mt0 + 1) * 128], qT[:, nsl],
                    start=True, stop=True,
                )
                nc.tensor.matmul(
                    pss[:, 512:], kT[:, mt1 * 128:(mt1 + 1) * 128], qT[:, nsl],
                    start=True, stop=True,
                )
                pT = p_pool.tile([128, 1024], BF)
                nc.scalar.activation(
                    pT[:], pss[:], mybir.ActivationFunctionType.Exp, scale=SCALE
                )
                pT_of[(j, p)] = pT

            def emit_epilogue(j):
                # epilogue: O = transpose(O^T) / transpose(rowsum)
                pso, psr = pso_of[j], psr_of[j]
                oT_sb = epi.tile([128, 512], F32)
                nc.vector.tensor_copy(oT_sb[:], pso[:])
                # DVE may read only one PSUM operand: stage row 32 via SBUF
                nc.vector.tensor_copy(rpad[32:33, :], psr[32:33, :])
                nc.vector.tensor_add(rpad[:1, :], psr[0:1, :], rpad[32:33, :])
                for t in range(4):
                    psot = ps.tile([128, 128], F32, tag="t", name="psot")
                    nc.tensor.transpose(
                        psot[:], oT_sb[:, t * 128:(t + 1) * 128], ident32[:]
                    )
                    psrt = ps.tile([128, 128], F32, tag="t", name="psrt")
                    nc.tensor.transpose(
                        psrt[:], rpad[:, t * 128:(t + 1) * 128], ident32[:]
                    )
                    rinv = outp.tile([128, 1], F32, name="rinv")
                    nc.vector.reciprocal(rinv[:], psrt[:, :1])
                    osb = outp.tile([128, 128], F32, name="osb")
                    nc.vector.tensor_scalar_mul(osb[:], psot[:], rinv[:])
                    row = j * 512 + t * 128
                    nc.sync.dma_start(out=out[row:row + 128, :], in_=osb[:])

            emit_s(*steps[0])
            emit_s(*steps[1])
            for i, (j, p) in enumerate(steps):
                if p == 0:
                    pso_of[j] = ps.tile([128, 512], F32, tag="o", bufs=1, name="pso")
                    psr_of[j] = ps.tile([128, 512], F32, tag="r", bufs=1, name="psr")
                pso, psr = pso_of[j], psr_of[j]
                pT = pT_of.pop((j, p))
                mt0, mt1 = 2 * p, 2 * p + 1
                nc.tensor.matmul(
                    pso[:], vv[:, mt0, :], pT[:, :512],
                    start=(p == 0), stop=False,
                )
                nc.tensor.matmul(
                    pso[:], vv[:, mt1, :], pT[:, 512:],
                    start=False, stop=(p == NP - 1),
                )
                nc.tensor.matmul(
                    psr[0:1, :], ones[:], pT[:, :512],
                    start=(p == 0), stop=(p == NP - 1),
                    tile_position=(0, 0),
                )
                nc.tensor.matmul(
                    psr[32:33, :], ones[:], pT[:, 512:],
                    start=(p == 0), stop=(p == NP - 1),
                    tile_position=(0, 32),
                )
                if i + 2 < len(steps):
                    emit_s(*steps[i + 2])
                if p == NP - 1:
                    emit_epilogue(j)

    nc.compile()
    return nc


_NC = None


def _get_nc():
    global _NC
    if _NC is None:
        _NC = _build()
    return _NC


def _in_maps(x, Wq, Wk, Wv):
    wqT = np.ascontiguousarray(np.asarray(Wq, np.float32).T).astype(NPBF)
    wkT = np.ascontiguousarray(np.asarray(Wk, np.float32).T).astype(NPBF)
    wvT = np.ascontiguousarray(np.asarray(Wv, np.float32).T).astype(NPBF)
    x = np.asarray(x, np.float32)
    maps = []
    for core in range(NCORES):
        b, half = core // 2, core % 2
        rolled = np.concatenate(
            [x[b, half * NQ:(half + 1) * NQ], x[b, (1 - half) * NQ:(2 - half) * NQ]],
            axis=0,
        )
        xT = np.ascontiguousarray(rolled.T).astype(NPBF)
        maps.append({"xT": xT, "wqT": wqT, "wkT": wkT, "wvT": wvT})
    return maps


def kernel(x, Wq, Wk, Wv):
    nc = _get_nc()
    maps = _in_maps(x, Wq, Wk, Wv)
    res = run_bass_kernel_spmd(nc, maps, list(range(NCORES)))
    out = np.empty((B, N, H), np.float32)
    for core in range(NCORES):
        b, half = core // 2, core % 2
        out[b, half * NQ:(half + 1) * NQ] = res.results[core]["out"]
    return out
